# revision 12
# baseline (speedup 1.0000x reference)
"""Trainium2 Bass kernel for the 4-way additive/bilinear/product/difference
attention module (B=64, T=256, H=768), data-parallel over batch across 8
NeuronCores.

Math per batch b (reference semantics):
  sc[i,j] = tanh((p@Wc2)[i,j] + (q@Wc1)[j,i]) * vc[i];  qc = softmax_j(sc) @ q
  sb[i,j] = (p@Wb@q^T)[i,j];                            qb = softmax_j(sb) @ q
  sd[i,j] = tanh(sum_h p[i,h]Wd[h]q[j,h]) * vd[j];      qd = softmax_j(sd) @ q
  sm[i,j] = tanh((q@Wm)[j] - (p@Wm)[i]) * vm[j];        qm = softmax_j(sm) @ q

Implementation notes:
- Score matrices are built TRANSPOSED (S^T[j,i]) so the attention matrix
  lands directly in the lhsT layout needed for the A@q matmul.
- The PE executes its queue strictly in order, so per-batch program order
  interleaves independent matmul work between every score-set and its
  attention_out consumer: the ACT/DVE exp-producer chains always have
  >1us of queued PE work to hide under.
- Matmuls in fp16 (PSUM accumulates fp32); bilinear path's attention
  matrix in bf16 with a global-max shift (see below).
- All inputs host-cast to device dtypes; output written f16, upcast on
  host. Halves HBM traffic vs f32 and removes SWDGE cast cost.
- qWm folded into the Wc1 matmul as a 257th column; pT carries a zeroed
  257th column so the wc2 accumulation covers the same PSUM region.
- vd/vm softmax-scale multiplies folded into the Exp activation's
  per-partition scale (ACT) — removes 4 DVE ops per batch.
- Activation/evac ops fused to [128,512] where layout allows (sd scores
  in one PSUM bank, sb exp in one op, sc exp/mult in one op).
- exp(sb - gmax) spans e^-70, below fp16 min subnormal -> bf16 for e_sb.
  The gmax chain (DVE->PE->DVE->PE->DVE) is staged between the two
  attention_out(0) halves so neither PE nor DVE ever block on it.
"""

import os

import numpy as np

B, T, H = 64, 256, 768
NCORES = 8
BPC = B // NCORES  # batches per core
HK = H // 128  # 6 h-chunks
TC = T // 128  # 2 t-chunks
NH = 384  # output free-dim half (PSUM bank limit: 512 f32)
HA = H + 1  # q augmented with a ones column (softmax denominator)
TA = T + 1  # Wc1 augmented with the Wm column (qWm fold); pT zero-padded

_CACHE = {}

# set by kernel() when BASS_KERNEL_TRACE=1 (read by test harness)
last_exec_time_ns = None
last_trace_dir = None


def _build_program():
    from contextlib import ExitStack

    import concourse.bass as bass
    import concourse.tile as tile
    from concourse import bacc, mybir
    from concourse.masks import make_identity

    f32 = mybir.dt.float32
    f16 = mybir.dt.float16
    bf16 = mybir.dt.bfloat16
    AF = mybir.ActivationFunctionType

    nc = bacc.Bacc(trn_type="TRN2")

    q_ext = nc.declare_dram_parameter("q16", [BPC, T, H], f16, isOutput=False)
    qb_ext = nc.declare_dram_parameter("qbf", [BPC, T, H], bf16, isOutput=False)
    p_ext = nc.declare_dram_parameter("p16", [BPC, T, H], f16, isOutput=False)
    wc1_ext = nc.declare_dram_parameter("Wc1a", [H, TA], f16, isOutput=False)
    wc2_ext = nc.declare_dram_parameter("Wc2", [H, T], f16, isOutput=False)
    vc_ext = nc.declare_dram_parameter("vc", [T, 1], f32, isOutput=False)
    wb_ext = nc.declare_dram_parameter("Wb", [H, H], f16, isOutput=False)
    wd_ext = nc.declare_dram_parameter("Wd", [H, 1], f32, isOutput=False)
    vd_ext = nc.declare_dram_parameter("vd", [T, 1], f32, isOutput=False)
    wm_ext = nc.declare_dram_parameter("Wm16", [H, 1], f16, isOutput=False)
    vm_ext = nc.declare_dram_parameter("vm", [T, 1], f32, isOutput=False)
    out_ext = nc.declare_dram_parameter("out", [4, BPC, T, H], f16, isOutput=True)

    def ap3(sl):
        # re-dimension a [T, H] dram AP into [128, TC, H] (strides in
        # elements): element (p, c, h) -> row c*128+p, col h
        return bass.AP(
            tensor=sl.tensor,
            offset=sl.offset,
            ap=[[H, 128], [128 * H, TC], [1, H]],
        )

    with tile.TileContext(nc) as tc, ExitStack() as ctx:
        const = ctx.enter_context(tc.tile_pool(name="const", bufs=1))
        io = ctx.enter_context(tc.tile_pool(name="io", bufs=3))
        trans = ctx.enter_context(tc.tile_pool(name="trans", bufs=2))
        epool = ctx.enter_context(tc.tile_pool(name="epool", bufs=3))
        small = ctx.enter_context(tc.tile_pool(name="small", bufs=4))
        # PSUM budget 8 banks (slots are bank-granular): psA(3: transpose
        # staging ring + sc score tiles + tiny broadcast tiles) + psb(2:
        # sb/sd raw scores, one [128,2,T] bank each) + pso(3: pwbT staging
        # + attention outputs) = 8. The single-tag rings are sequenced so
        # every reallocation's WAR lands on a long-completed reader.
        psA = ctx.enter_context(tc.tile_pool(name="psA", bufs=3, space="PSUM"))
        pstr = psA
        pstiny = psA
        ps256 = psA
        psb = ctx.enter_context(tc.tile_pool(name="psb", bufs=2, space="PSUM"))
        pso = ctx.enter_context(tc.tile_pool(name="pso", bufs=3, space="PSUM"))

        # ---- tiny constants (no DRAM dependency) ----
        ident = const.tile([128, 128], f16, tag="ident")
        make_identity(nc, ident)
        ones_row = const.tile([1, 128], f16, tag="ones_row")
        nc.vector.memset(ones_row, 1.0)

        # ---- input loads: one 3D DMA per tensor per batch ----
        def load_batch(b, engs=None):
            qn = io.tile([128, TC, HA], f16, tag="qn", name=f"qn_{b}")
            pn = io.tile([128, TC, H], f16, tag="pn", name=f"pn_{b}")
            qn_bf = io.tile([128, TC, HA], bf16, tag="qn_bf", name=f"qnbf_{b}")
            e1, e2, e3 = engs or (nc.gpsimd, nc.gpsimd, nc.gpsimd)
            e1.dma_start(out=qn[:, :, 0:H], in_=ap3(q_ext[b]))
            nc.vector.memset(qn[:, :, H : H + 1], 1.0)
            e2.dma_start(out=pn, in_=ap3(p_ext[b]))
            e3.dma_start(out=qn_bf[:, :, 0:H], in_=ap3(qb_ext[b]))
            nc.vector.memset(qn_bf[:, :, H : H + 1], 1.0)
            return qn, pn, qn_bf

        pre = {0: load_batch(0, engs=(nc.sync, nc.scalar, nc.gpsimd))}

        # ---- weights: wc1/wc2 first (sc needs them ~5us in), wb next
        # (pwbT), split across the two HWDGE rings; small vectors on
        # SWDGE. ----
        wc1 = []
        wc2 = []
        wb = []
        wd = []
        wm = []
        for k in range(HK):
            t1 = const.tile([128, TA], f16, tag=f"wc1_{k}", name=f"wc1_{k}")
            nc.sync.dma_start(out=t1, in_=wc1_ext[128 * k : 128 * (k + 1), :])
            wc1.append(t1)
            t2 = const.tile([128, T], f16, tag=f"wc2_{k}", name=f"wc2_{k}")
            nc.scalar.dma_start(out=t2, in_=wc2_ext[128 * k : 128 * (k + 1), :])
            wc2.append(t2)
        for k in range(HK):
            t3 = const.tile([128, H], f16, tag=f"wb_{k}", name=f"wb_{k}")
            eng = nc.sync if k % 2 == 0 else nc.scalar
            eng.dma_start(out=t3, in_=wb_ext[128 * k : 128 * (k + 1), :])
            wb.append(t3)
        for k in range(HK):
            t4 = const.tile([128, 1], f32, tag=f"wd_{k}", name=f"wd_{k}")
            nc.gpsimd.dma_start(out=t4, in_=wd_ext[128 * k : 128 * (k + 1), :])
            wd.append(t4)
            t5 = const.tile([128, 1], f16, tag=f"wm_{k}", name=f"wm_{k}")
            nc.gpsimd.dma_start(out=t5, in_=wm_ext[128 * k : 128 * (k + 1), :])
            wm.append(t5)
        vd_c = []
        vm_c = []
        for c in range(TC):
            t6 = const.tile([128, 1], f32, tag=f"vd_{c}", name=f"vd_{c}")
            nc.gpsimd.dma_start(out=t6, in_=vd_ext[128 * c : 128 * (c + 1), :])
            vd_c.append(t6)
            t7 = const.tile([128, 1], f32, tag=f"vm_{c}", name=f"vm_{c}")
            nc.gpsimd.dma_start(out=t7, in_=vm_ext[128 * c : 128 * (c + 1), :])
            vm_c.append(t7)
        # vc broadcast across partitions, duplicated for both jc halves:
        # vc_bc2[p, jc, i] = vc[i]
        vc_bc2 = const.tile([128, TC, T], f32, tag="vc_bc2")
        vcf = vc_ext[:, 0]
        nc.gpsimd.dma_start(
            out=vc_bc2,
            in_=bass.AP(
                tensor=vcf.tensor, offset=vcf.offset, ap=[[0, 128], [0, TC]] + vcf.ap
            ),
        )
        pre[1] = load_batch(1)

        # ---- per-batch body ----
        for b in range(BPC):
            qn, pn, qn_bf = pre.pop(b) if b in pre else load_batch(b)

            # --- transposes on TensorE; four 128x128 blocks share one PSUM
            # staging tile -> single wide [128, 512] evacuation ---
            qT = trans.tile([128, HK, T], f16, tag="qT", name=f"qT_{b}")
            pT = trans.tile([128, HK, TA], f16, tag="pT", name=f"pT_{b}")
            pdT = trans.tile([128, HK, T], f16, tag="pdT", name=f"pdT_{b}")
            nc.vector.memset(pT[:, :, T:TA], 0.0)
            cidx = 0
            for src_, dst_ in ((qn, qT), (pn, pT)):
                for kh in range(HK // 2):
                    tq = pstr.tile([128, 2, T], f16, tag="pstr", name=f"t_{b}_{cidx}")
                    for s in range(2):
                        k = 2 * kh + s
                        for c in range(TC):
                            nc.tensor.transpose(
                                tq[:, s, 128 * c : 128 * (c + 1)],
                                src_[:, c, 128 * k : 128 * (k + 1)],
                                ident,
                            )
                    dst_sl = (
                        dst_[:, 2 * kh : 2 * kh + 2, :]
                        if dst_ is qT
                        else dst_[:, 2 * kh : 2 * kh + 2, 0:T]
                    )
                    if cidx % 3 != 2:
                        nc.vector.tensor_copy(dst_sl, tq)
                    else:
                        nc.scalar.copy(dst_sl, tq)
                    cidx += 1
            for k in range(HK):
                # pdT = pT * Wd[h] (per-partition scalar) on GPSIMD
                # (SBUF->SBUF; keeps DVE/ACT free for PSUM evacuations)
                nc.gpsimd.tensor_scalar_mul(pdT[:, k, :], pT[:, k, 0:T], wd[k])

            # --- sc scores (transposed layout), qWm rides along as col 256 ---
            e_sc = epool.tile([128, TC, T], f16, tag="e", name=f"esc_{b}")
            tmp_sc = epool.tile([128, TC, T], f32, tag="tmp", name=f"tsc_{b}")
            qwm_sb = []
            for jc in range(TC):
                ups = ps256.tile([128, TA], f32, tag="pstr", name=f"usc_{b}_{jc}")
                for k in range(HK):
                    nc.tensor.matmul(
                        ups,
                        qT[:, k, 128 * jc : 128 * (jc + 1)],
                        wc1[k],
                        start=(k == 0),
                        stop=False,
                    )
                for k in range(HK):
                    nc.tensor.matmul(
                        ups,
                        wc2[k][:, 128 * jc : 128 * (jc + 1)],
                        pT[:, k, :],
                        start=False,
                        stop=(k == HK - 1),
                    )
                qcol = small.tile([128, 1], f32, tag="qwm", name=f"qwm_{b}_{jc}")
                nc.vector.tensor_copy(qcol, ups[:, T:TA])
                qwm_sb.append(qcol)
                nc.scalar.activation(tmp_sc[:, jc, :], ups[:, 0:T], AF.Tanh)
            nc.gpsimd.tensor_mul(tmp_sc, tmp_sc, vc_bc2)
            nc.scalar.activation(e_sc, tmp_sc, AF.Exp)

            # --- pwbT[h', i] = sum_h Wb[h, h'] * pT[h, i] ---
            pwbT = trans.tile([128, HK, T], f16, tag="pwbT", name=f"pwbT_{b}")
            for k2 in range(HK):
                pws = pso.tile([128, T], f32, tag="pso", name=f"pws_{b}_{k2}")
                for k in range(HK):
                    nc.tensor.matmul(
                        pws,
                        wb[k][:, 128 * k2 : 128 * (k2 + 1)],
                        pT[:, k, 0:T],
                        start=(k == 0),
                        stop=(k == HK - 1),
                    )
                if k2 % 2 == 0:
                    nc.vector.tensor_copy(pwbT[:, k2, :], pws)
                else:
                    nc.scalar.copy(pwbT[:, k2, :], pws)

            # --- sb raw scores into a single PSUM bank [128, 2, T] ---
            sbps = psb.tile([128, TC, T], f32, tag="psb", name=f"sb_{b}")
            for jc in range(TC):
                for k2 in range(HK):
                    nc.tensor.matmul(
                        sbps[:, jc, :],
                        qT[:, k2, 128 * jc : 128 * (jc + 1)],
                        pwbT[:, k2, :],
                        start=(k2 == 0),
                        stop=(k2 == HK - 1),
                    )
            m0 = small.tile([128, 1], f16, tag="m0", name=f"m0_{b}")
            m1 = small.tile([128, 1], f16, tag="m1", name=f"m1_{b}")
            nc.vector.reduce_max(m0, sbps[:, 0, :], axis=mybir.AxisListType.X)
            nc.vector.reduce_max(m1, sbps[:, 1, :], axis=mybir.AxisListType.X)
            nc.vector.tensor_max(m0, m0, m1)

            # --- attention_out helper: one ic-half of one attention ---
            def attn_ic(att, e, rhs_qn, ic, osb2):
                zrec = small.tile([128, 1], f32, tag="zrec", name=f"zr_{att}_{b}_{ic}")
                for nh in (1, 0):
                    w = (HA - NH) if nh == 1 else NH  # 385 or 384
                    ops = pso.tile(
                        [128, 512], f32, tag="pso", name=f"o_{att}_{b}_{ic}_{nh}"
                    )
                    for jc in range(TC):
                        nc.tensor.matmul(
                            ops[:, 0:w],
                            e[:, jc, 128 * ic : 128 * (ic + 1)],
                            rhs_qn[:, jc, NH * nh : NH * nh + w],
                            start=(jc == 0),
                            stop=(jc == TC - 1),
                        )
                    if nh == 1:
                        nc.vector.reciprocal(zrec, ops[:, 384:385])
                    dst = osb2[:, ic, NH * nh : NH * (nh + 1)]
                    if att == 1 or (att == 3 and nh == 1):
                        nc.scalar.activation(dst, ops[:, 0:NH], AF.Copy, scale=zrec)
                    else:
                        nc.vector.tensor_scalar_mul(dst, ops[:, 0:NH], zrec)

            def attn_dma(att, osb2):
                nc.sync.dma_start(out=ap3(out_ext[att, b]), in_=osb2)

            # --- attention_out(0) with the gmax chain staged between the
            # two ic halves (PE never waits on DVE and vice versa) ---
            osb0 = epool.tile([128, TC, H], f16, tag="osb", name=f"osb0_{b}")
            attn_ic(0, e_sc, qn, 0, osb0)
            mt = pstiny.tile([1, 128], f16, tag="pstr", name=f"mt_{b}")
            nc.tensor.transpose(mt, m0, ident)
            gneg = small.tile([1, 1], f16, tag="gneg", name=f"g_{b}")
            nc.vector.reduce_max(gneg, mt, axis=mybir.AxisListType.X)
            nc.vector.tensor_scalar_mul(gneg, gneg, -1.0)
            attn_ic(0, e_sc, qn, 1, osb0)
            attn_dma(0, osb0)

            # --- sd raw scores into a single PSUM bank; gnps between the
            # two jc halves ---
            dps = psb.tile([128, TC, T], f32, tag="psb", name=f"sd_{b}")
            for k in range(HK):
                nc.tensor.matmul(
                    dps[:, 0, :],
                    qT[:, k, 0:128],
                    pdT[:, k, :],
                    start=(k == 0),
                    stop=(k == HK - 1),
                )
            gnps = pstiny.tile([128, 1], f32, tag="pstr", name=f"gnps_{b}")
            nc.tensor.matmul(gnps, ones_row, gneg, start=True, stop=True)
            gnb = small.tile([128, 1], f32, tag="gnb", name=f"gnb_{b}")
            nc.vector.tensor_copy(gnb, gnps)
            for k in range(HK):
                nc.tensor.matmul(
                    dps[:, 1, :],
                    qT[:, k, 128:256],
                    pdT[:, k, :],
                    start=(k == 0),
                    stop=(k == HK - 1),
                )
            # e_sb = exp(sb - gmax), one wide op, bf16 (range)
            e_sb = epool.tile([128, TC, T], bf16, tag="e_bf", name=f"esb_{b}")
            nc.scalar.activation(e_sb, sbps, AF.Exp, bias=gnb)

            # --- sm: pwm row + broadcast (independent PE work to hide the
            # e_sb exp under) ---
            pws2 = pstiny.tile([1, T], f32, tag="pstr", name=f"pwm_{b}")
            for k in range(HK):
                nc.tensor.matmul(
                    pws2, wm[k], pT[:, k, 0:T], start=(k == 0), stop=(k == HK - 1)
                )
            pwm_row = small.tile([1, T], f16, tag="pwm_row", name=f"pwmr_{b}")
            nc.vector.tensor_copy(pwm_row, pws2)
            pwm_bc = pstiny.tile([128, T], f32, tag="pstr", name=f"pwmb_{b}")
            nc.tensor.matmul(pwm_bc, ones_row, pwm_row, start=True, stop=True)

            # sd producer: one wide tanh, exp with vd folded into scale
            tmp_sd = epool.tile([128, TC, T], f32, tag="tmp", name=f"tsd_{b}")
            nc.scalar.activation(tmp_sd, dps, AF.Tanh)
            e_sd = epool.tile([128, TC, T], f16, tag="e", name=f"esd_{b}")
            for jc in range(TC):
                nc.scalar.activation(
                    e_sd[:, jc, :], tmp_sd[:, jc, :], AF.Exp, scale=vd_c[jc]
                )

            # --- attention_out(1) (bilinear) ---
            osb1 = epool.tile([128, TC, H], f16, tag="osb", name=f"osb1_{b}")
            attn_ic(1, e_sb, qn_bf, 0, osb1)
            attn_ic(1, e_sb, qn_bf, 1, osb1)
            attn_dma(1, osb1)

            # sm producer: tanh(qwm[j] - pwm[i]), exp with vm folded
            tmp_sm = epool.tile([128, TC, T], f32, tag="tmp", name=f"tsm_{b}")
            e_sm = epool.tile([128, TC, T], f16, tag="e", name=f"esm_{b}")
            for jc in range(TC):
                nc.scalar.activation(
                    tmp_sm[:, jc, :], pwm_bc, AF.Tanh, bias=qwm_sb[jc], scale=-1.0
                )
                nc.scalar.activation(
                    e_sm[:, jc, :], tmp_sm[:, jc, :], AF.Exp, scale=vm_c[jc]
                )

            # --- attention_out(2) (product) ---
            osb2_ = epool.tile([128, TC, H], f16, tag="osb", name=f"osb2_{b}")
            attn_ic(2, e_sd, qn, 0, osb2_)
            attn_ic(2, e_sd, qn, 1, osb2_)
            attn_dma(2, osb2_)

            # --- attention_out(3) (difference) ---
            osb3 = epool.tile([128, TC, H], f16, tag="osb", name=f"osb3_{b}")
            attn_ic(3, e_sm, qn, 0, osb3)
            attn_ic(3, e_sm, qn, 1, osb3)
            attn_dma(3, osb3)

            if b + 2 < BPC and b + 2 not in pre:
                pre[b + 2] = load_batch(b + 2)

    nc.compile()
    return nc


def _get_program():
    if "nc" not in _CACHE:
        _CACHE["nc"] = _build_program()
    return _CACHE["nc"]


def kernel(**inputs):
    global last_exec_time_ns, last_trace_dir
    import ml_dtypes
    from concourse.bass_utils import run_bass_kernel_spmd

    nc = _get_program()

    f32 = lambda k: np.ascontiguousarray(np.asarray(inputs[k], dtype=np.float32))
    q32 = f32("q")
    p32 = f32("p")
    q16 = q32.astype(np.float16)
    qbf = q32.astype(ml_dtypes.bfloat16)
    p16 = p32.astype(np.float16)
    wc1a = np.ascontiguousarray(
        np.concatenate([f32("Wc1"), f32("Wm")], axis=1).astype(np.float16)
    )
    weights = {
        "Wc1a": wc1a,
        "Wc2": f32("Wc2").astype(np.float16),
        "Wb": f32("Wb").astype(np.float16),
        "Wm16": f32("Wm").astype(np.float16),
        "vc": f32("vc"),
        "Wd": f32("Wd"),
        "vd": f32("vd"),
        "vm": f32("vm"),
    }

    in_maps = []
    for i in range(NCORES):
        m = {
            "q16": q16[i * BPC : (i + 1) * BPC],
            "qbf": qbf[i * BPC : (i + 1) * BPC],
            "p16": p16[i * BPC : (i + 1) * BPC],
        }
        m.update(weights)
        in_maps.append(m)

    trace = bool(int(os.environ.get("BASS_KERNEL_TRACE", "0")))
    kw = {}
    if trace:
        kw.update(trace=True)
        tmpdir = os.environ.get("BASS_KERNEL_TRACE_DIR")
        if tmpdir:
            os.makedirs(tmpdir, exist_ok=True)
            kw.update(tmpdir=tmpdir)
    res = run_bass_kernel_spmd(nc, in_maps, core_ids=list(range(NCORES)), **kw)
    last_exec_time_ns = getattr(res, "exec_time_ns", None)
    results = res.results

    outs = [np.empty((B, T, H), dtype=np.float32) for _ in range(4)]
    for i in range(NCORES):
        o = np.asarray(results[i]["out"], dtype=np.float32)
        for a in range(4):
            outs[a][i * BPC : (i + 1) * BPC] = o[a]
    return tuple(outs)


# revision 14
# speedup vs baseline: 1.8361x; 1.8361x over previous
"""Trainium2 Bass kernel for the 4-way additive/bilinear/product/difference
attention module (B=64, T=256, H=768), data-parallel over batch across 8
NeuronCores.

Math per batch b (reference semantics):
  sc[i,j] = tanh((p@Wc2)[i,j] + (q@Wc1)[j,i]) * vc[i];  qc = softmax_j(sc) @ q
  sb[i,j] = (p@Wb@q^T)[i,j];                            qb = softmax_j(sb) @ q
  sd[i,j] = tanh(sum_h p[i,h]Wd[h]q[j,h]) * vd[j];      qd = softmax_j(sd) @ q
  sm[i,j] = tanh((q@Wm)[j] - (p@Wm)[i]) * vm[j];        qm = softmax_j(sm) @ q

Implementation notes:
- Score matrices are built TRANSPOSED (S^T[j,i]) so the attention matrix
  lands directly in the lhsT layout needed for the A@q matmul.
- The PE executes its queue strictly in order, so per-batch program order
  interleaves independent matmul work between every score-set and its
  attention_out consumer: the ACT/DVE exp-producer chains always have
  >1us of queued PE work to hide under.
- Matmuls in fp16 (PSUM accumulates fp32); bilinear path's attention
  matrix in bf16 with a global-max shift (see below).
- All inputs host-cast to device dtypes; output written f16, upcast on
  host. Halves HBM traffic vs f32 and removes SWDGE cast cost.
- qWm folded into the Wc1 matmul as a 257th column; pT carries a zeroed
  257th column so the wc2 accumulation covers the same PSUM region.
- vd/vm softmax-scale multiplies folded into the Exp activation's
  per-partition scale (ACT) — removes 4 DVE ops per batch.
- Activation/evac ops fused to [128,512] where layout allows (sd scores
  in one PSUM bank, sb exp in one op, sc exp/mult in one op).
- exp(sb - gmax) spans e^-70, below fp16 min subnormal -> bf16 for e_sb.
  The gmax chain (DVE->PE->DVE->PE->DVE) is staged between the two
  attention_out(0) halves so neither PE nor DVE ever block on it.
"""

import os

import numpy as np

B, T, H = 64, 256, 768
NCORES = 8
BPC = B // NCORES  # batches per core
HK = H // 128  # 6 h-chunks
TC = T // 128  # 2 t-chunks
NH = 384  # output free-dim half (PSUM bank limit: 512 f32)
HA = H + 1  # q augmented with a ones column (softmax denominator)
TA = T + 1  # Wc1 augmented with the Wm column (qWm fold); pT zero-padded

_CACHE = {}

# set by kernel() when BASS_KERNEL_TRACE=1 (read by test harness)
last_exec_time_ns = None
last_trace_dir = None


def _build_program():
    from contextlib import ExitStack

    import concourse.bass as bass
    import concourse.tile as tile
    from concourse import bacc, mybir
    from concourse.masks import make_identity

    f32 = mybir.dt.float32
    f16 = mybir.dt.float16
    bf16 = mybir.dt.bfloat16
    AF = mybir.ActivationFunctionType

    nc = bacc.Bacc(trn_type="TRN2")

    q_ext = nc.declare_dram_parameter("q16", [BPC, T, H], f16, isOutput=False)
    qb_ext = nc.declare_dram_parameter("qbf", [BPC, T, H], bf16, isOutput=False)
    p_ext = nc.declare_dram_parameter("p16", [BPC, T, H], f16, isOutput=False)
    wc1_ext = nc.declare_dram_parameter("Wc1a", [H, TA], f16, isOutput=False)
    wc2_ext = nc.declare_dram_parameter("Wc2", [H, T], f16, isOutput=False)
    vc_ext = nc.declare_dram_parameter("vc", [T, 1], f32, isOutput=False)
    wb_ext = nc.declare_dram_parameter("Wb", [H, H], f16, isOutput=False)
    wd_ext = nc.declare_dram_parameter("Wd", [H, 1], f32, isOutput=False)
    vd_ext = nc.declare_dram_parameter("vd", [T, 1], f32, isOutput=False)
    wm_ext = nc.declare_dram_parameter("Wm16", [H, 1], f16, isOutput=False)
    vm_ext = nc.declare_dram_parameter("vm", [T, 1], f32, isOutput=False)
    out_ext = nc.declare_dram_parameter("out", [4, BPC, T, H], f16, isOutput=True)

    def ap3(sl):
        # re-dimension a [T, H] dram AP into [128, TC, H] (strides in
        # elements): element (p, c, h) -> row c*128+p, col h
        return bass.AP(
            tensor=sl.tensor,
            offset=sl.offset,
            ap=[[H, 128], [128 * H, TC], [1, H]],
        )

    with tile.TileContext(nc) as tc, ExitStack() as ctx:
        const = ctx.enter_context(tc.tile_pool(name="const", bufs=1))
        io = ctx.enter_context(tc.tile_pool(name="io", bufs=3))
        trans = ctx.enter_context(tc.tile_pool(name="trans", bufs=2))
        epool = ctx.enter_context(tc.tile_pool(name="epool", bufs=3))
        small = ctx.enter_context(tc.tile_pool(name="small", bufs=4))
        # PSUM budget 8 banks (slots are bank-granular): psA(3: transpose
        # staging ring + sc score tiles + tiny broadcast tiles) + psb(2:
        # sb/sd raw scores, one [128,2,T] bank each) + pso(3: pwbT staging
        # + attention outputs) = 8. The single-tag rings are sequenced so
        # every reallocation's WAR lands on a long-completed reader.
        psA = ctx.enter_context(tc.tile_pool(name="psA", bufs=3, space="PSUM"))
        pstr = psA
        pstiny = psA
        ps256 = psA
        psb = ctx.enter_context(tc.tile_pool(name="psb", bufs=2, space="PSUM"))
        pso = ctx.enter_context(tc.tile_pool(name="pso", bufs=3, space="PSUM"))

        # ---- tiny constants (no DRAM dependency) ----
        ident = const.tile([128, 128], f16, tag="ident")
        make_identity(nc, ident)
        ones_row = const.tile([1, 128], f16, tag="ones_row")
        nc.vector.memset(ones_row, 1.0)

        # ---- input loads: one 3D DMA per tensor per batch ----
        def load_batch(b, engs=None):
            qn = io.tile([128, TC, HA], f16, tag="qn", name=f"qn_{b}")
            pn = io.tile([128, TC, H], f16, tag="pn", name=f"pn_{b}")
            qn_bf = io.tile([128, TC, HA], bf16, tag="qn_bf", name=f"qnbf_{b}")
            e1, e2, e3 = engs or (nc.gpsimd, nc.gpsimd, nc.gpsimd)
            e1.dma_start(out=qn[:, :, 0:H], in_=ap3(q_ext[b]))
            nc.vector.memset(qn[:, :, H : H + 1], 1.0)
            e2.dma_start(out=pn, in_=ap3(p_ext[b]))
            e3.dma_start(out=qn_bf[:, :, 0:H], in_=ap3(qb_ext[b]))
            nc.vector.memset(qn_bf[:, :, H : H + 1], 1.0)
            return qn, pn, qn_bf

        pre = {0: load_batch(0, engs=(nc.sync, nc.scalar, nc.gpsimd))}

        # ---- weights: wc1/wc2 first (sc needs them ~5us in), wb next
        # (pwbT), split across the two HWDGE rings; small vectors on
        # SWDGE. ----
        wc1 = []
        wc2 = []
        wb = []
        wd = []
        wm = []
        for k in range(HK):
            t1 = const.tile([128, TA], f16, tag=f"wc1_{k}", name=f"wc1_{k}")
            nc.sync.dma_start(out=t1, in_=wc1_ext[128 * k : 128 * (k + 1), :])
            wc1.append(t1)
            t2 = const.tile([128, T], f16, tag=f"wc2_{k}", name=f"wc2_{k}")
            nc.scalar.dma_start(out=t2, in_=wc2_ext[128 * k : 128 * (k + 1), :])
            wc2.append(t2)
        for k in range(HK):
            t3 = const.tile([128, H], f16, tag=f"wb_{k}", name=f"wb_{k}")
            eng = nc.sync if k % 2 == 0 else nc.scalar
            eng.dma_start(out=t3, in_=wb_ext[128 * k : 128 * (k + 1), :])
            wb.append(t3)
        for k in range(HK):
            t4 = const.tile([128, 1], f32, tag=f"wd_{k}", name=f"wd_{k}")
            nc.gpsimd.dma_start(out=t4, in_=wd_ext[128 * k : 128 * (k + 1), :])
            wd.append(t4)
            t5 = const.tile([128, 1], f16, tag=f"wm_{k}", name=f"wm_{k}")
            nc.gpsimd.dma_start(out=t5, in_=wm_ext[128 * k : 128 * (k + 1), :])
            wm.append(t5)
        vd_c = []
        vm_c = []
        for c in range(TC):
            t6 = const.tile([128, 1], f32, tag=f"vd_{c}", name=f"vd_{c}")
            nc.gpsimd.dma_start(out=t6, in_=vd_ext[128 * c : 128 * (c + 1), :])
            vd_c.append(t6)
            t7 = const.tile([128, 1], f32, tag=f"vm_{c}", name=f"vm_{c}")
            nc.gpsimd.dma_start(out=t7, in_=vm_ext[128 * c : 128 * (c + 1), :])
            vm_c.append(t7)
        # vc broadcast across partitions, duplicated for both jc halves:
        # vc_bc2[p, jc, i] = vc[i]
        vc_bc2 = const.tile([128, TC, T], f32, tag="vc_bc2")
        vcf = vc_ext[:, 0]
        nc.gpsimd.dma_start(
            out=vc_bc2,
            in_=bass.AP(
                tensor=vcf.tensor, offset=vcf.offset, ap=[[0, 128], [0, TC]] + vcf.ap
            ),
        )
        pre[1] = load_batch(1)

        # ---- per-batch body ----
        for b in range(BPC):
            qn, pn, qn_bf = pre.pop(b) if b in pre else load_batch(b)

            # --- transposes on TensorE; four 128x128 blocks share one PSUM
            # staging tile -> single wide [128, 512] evacuation ---
            qT = trans.tile([128, HK, T], f16, tag="qT", name=f"qT_{b}")
            pT = trans.tile([128, HK, TA], f16, tag="pT", name=f"pT_{b}")
            pdT = trans.tile([128, HK, T], f16, tag="pdT", name=f"pdT_{b}")
            nc.vector.memset(pT[:, :, T:TA], 0.0)
            cidx = 0
            for src_, dst_ in ((qn, qT), (pn, pT)):
                for kh in range(HK // 2):
                    tq = pstr.tile([128, 2, T], f16, tag="pstr", name=f"t_{b}_{cidx}")
                    for s in range(2):
                        k = 2 * kh + s
                        for c in range(TC):
                            nc.tensor.transpose(
                                tq[:, s, 128 * c : 128 * (c + 1)],
                                src_[:, c, 128 * k : 128 * (k + 1)],
                                ident,
                            )
                    dst_sl = (
                        dst_[:, 2 * kh : 2 * kh + 2, :]
                        if dst_ is qT
                        else dst_[:, 2 * kh : 2 * kh + 2, 0:T]
                    )
                    if cidx % 3 != 2:
                        nc.vector.tensor_copy(dst_sl, tq)
                    else:
                        nc.scalar.copy(dst_sl, tq)
                    cidx += 1
            for k in range(HK):
                # pdT = pT * Wd[h] (per-partition scalar), split DVE/ACT
                if k % 2 == 0:
                    nc.vector.tensor_scalar_mul(pdT[:, k, :], pT[:, k, 0:T], wd[k])
                else:
                    nc.scalar.activation(pdT[:, k, :], pT[:, k, 0:T], AF.Copy, scale=wd[k])

            # --- sc scores (transposed layout), qWm rides along as col 256 ---
            e_sc = epool.tile([128, TC, T], f16, tag="e", name=f"esc_{b}")
            tmp_sc = epool.tile([128, TC, T], f32, tag="tmp", name=f"tsc_{b}")
            qwm_sb = []
            for jc in range(TC):
                ups = ps256.tile([128, TA], f32, tag="pstr", name=f"usc_{b}_{jc}")
                for k in range(HK):
                    nc.tensor.matmul(
                        ups,
                        qT[:, k, 128 * jc : 128 * (jc + 1)],
                        wc1[k],
                        start=(k == 0),
                        stop=False,
                    )
                for k in range(HK):
                    nc.tensor.matmul(
                        ups,
                        wc2[k][:, 128 * jc : 128 * (jc + 1)],
                        pT[:, k, :],
                        start=False,
                        stop=(k == HK - 1),
                    )
                qcol = small.tile([128, 1], f32, tag="qwm", name=f"qwm_{b}_{jc}")
                nc.vector.tensor_copy(qcol, ups[:, T:TA])
                qwm_sb.append(qcol)
                nc.scalar.activation(tmp_sc[:, jc, :], ups[:, 0:T], AF.Tanh)
            nc.vector.tensor_mul(tmp_sc, tmp_sc, vc_bc2)
            # per-jc exp writes: a single full-tile ACT write read by PE
            # LDWEIGHTS raced (wait computed against the ring slot's
            # previous writer) — keep sliced writes, they sync correctly
            for jc in range(TC):
                nc.scalar.activation(e_sc[:, jc, :], tmp_sc[:, jc, :], AF.Exp)

            # --- pwbT[h', i] = sum_h Wb[h, h'] * pT[h, i] ---
            pwbT = trans.tile([128, HK, T], f16, tag="pwbT", name=f"pwbT_{b}")
            for k2 in range(HK):
                pws = pso.tile([128, T], f32, tag="pso", name=f"pws_{b}_{k2}")
                for k in range(HK):
                    nc.tensor.matmul(
                        pws,
                        wb[k][:, 128 * k2 : 128 * (k2 + 1)],
                        pT[:, k, 0:T],
                        start=(k == 0),
                        stop=(k == HK - 1),
                    )
                if k2 % 2 == 0:
                    nc.vector.tensor_copy(pwbT[:, k2, :], pws)
                else:
                    nc.scalar.copy(pwbT[:, k2, :], pws)

            # --- sb raw scores into a single PSUM bank [128, 2, T] ---
            sbps = psb.tile([128, TC, T], f32, tag="psb", name=f"sb_{b}")
            for jc in range(TC):
                for k2 in range(HK):
                    nc.tensor.matmul(
                        sbps[:, jc, :],
                        qT[:, k2, 128 * jc : 128 * (jc + 1)],
                        pwbT[:, k2, :],
                        start=(k2 == 0),
                        stop=(k2 == HK - 1),
                    )
            m0 = small.tile([128, 1], f16, tag="m0", name=f"m0_{b}")
            m1 = small.tile([128, 1], f16, tag="m1", name=f"m1_{b}")
            nc.vector.reduce_max(m0, sbps[:, 0, :], axis=mybir.AxisListType.X)
            nc.vector.reduce_max(m1, sbps[:, 1, :], axis=mybir.AxisListType.X)
            nc.vector.tensor_max(m0, m0, m1)

            # --- attention_out helper: one ic-half of one attention ---
            def attn_ic(att, e, rhs_qn, ic, osb2):
                zrec = small.tile([128, 1], f32, tag="zrec", name=f"zr_{att}_{b}_{ic}")
                for nh in (1, 0):
                    w = (HA - NH) if nh == 1 else NH  # 385 or 384
                    ops = pso.tile(
                        [128, 512], f32, tag="pso", name=f"o_{att}_{b}_{ic}_{nh}"
                    )
                    for jc in range(TC):
                        nc.tensor.matmul(
                            ops[:, 0:w],
                            e[:, jc, 128 * ic : 128 * (ic + 1)],
                            rhs_qn[:, jc, NH * nh : NH * nh + w],
                            start=(jc == 0),
                            stop=(jc == TC - 1),
                        )
                    if nh == 1:
                        nc.vector.reciprocal(zrec, ops[:, 384:385])
                    dst = osb2[:, ic, NH * nh : NH * (nh + 1)]
                    if att == 1 or (att == 3 and nh == 1):
                        nc.scalar.activation(dst, ops[:, 0:NH], AF.Copy, scale=zrec)
                    else:
                        nc.vector.tensor_scalar_mul(dst, ops[:, 0:NH], zrec)

            def attn_dma(att, osb2):
                nc.sync.dma_start(out=ap3(out_ext[att, b]), in_=osb2)

            # --- attention_out(0) with the gmax chain staged between the
            # two ic halves (PE never waits on DVE and vice versa) ---
            osb0 = epool.tile([128, TC, H], f16, tag="osb", name=f"osb0_{b}")
            attn_ic(0, e_sc, qn, 0, osb0)
            mt = pstiny.tile([1, 128], f16, tag="pstr", name=f"mt_{b}")
            nc.tensor.transpose(mt, m0, ident)
            gneg = small.tile([1, 1], f16, tag="gneg", name=f"g_{b}")
            nc.vector.reduce_max(gneg, mt, axis=mybir.AxisListType.X)
            nc.vector.tensor_scalar_mul(gneg, gneg, -1.0)
            attn_ic(0, e_sc, qn, 1, osb0)
            attn_dma(0, osb0)

            # --- sd raw scores into a single PSUM bank; gnps between the
            # two jc halves ---
            dps = psb.tile([128, TC, T], f32, tag="psb", name=f"sd_{b}")
            for k in range(HK):
                nc.tensor.matmul(
                    dps[:, 0, :],
                    qT[:, k, 0:128],
                    pdT[:, k, :],
                    start=(k == 0),
                    stop=(k == HK - 1),
                )
            gnps = pstiny.tile([128, 1], f32, tag="pstr", name=f"gnps_{b}")
            nc.tensor.matmul(gnps, ones_row, gneg, start=True, stop=True)
            gnb = small.tile([128, 1], f32, tag="gnb", name=f"gnb_{b}")
            nc.vector.tensor_copy(gnb, gnps)
            for k in range(HK):
                nc.tensor.matmul(
                    dps[:, 1, :],
                    qT[:, k, 128:256],
                    pdT[:, k, :],
                    start=(k == 0),
                    stop=(k == HK - 1),
                )
            # e_sb = exp(sb - gmax), bf16 (range); per-jc writes (see
            # e_sc race note)
            e_sb = epool.tile([128, TC, T], bf16, tag="e_bf", name=f"esb_{b}")
            for jc in range(TC):
                nc.scalar.activation(e_sb[:, jc, :], sbps[:, jc, :], AF.Exp, bias=gnb)

            # --- sm: pwm row + broadcast (independent PE work to hide the
            # e_sb exp under) ---
            pws2 = pstiny.tile([1, T], f32, tag="pstr", name=f"pwm_{b}")
            for k in range(HK):
                nc.tensor.matmul(
                    pws2, wm[k], pT[:, k, 0:T], start=(k == 0), stop=(k == HK - 1)
                )
            pwm_row = small.tile([1, T], f16, tag="pwm_row", name=f"pwmr_{b}")
            nc.vector.tensor_copy(pwm_row, pws2)
            pwm_bc = pstiny.tile([128, T], f32, tag="pstr", name=f"pwmb_{b}")
            nc.tensor.matmul(pwm_bc, ones_row, pwm_row, start=True, stop=True)

            # sd producer: one wide tanh, exp with vd folded into scale
            tmp_sd = epool.tile([128, TC, T], f32, tag="tmp", name=f"tsd_{b}")
            nc.scalar.activation(tmp_sd, dps, AF.Tanh)
            e_sd = epool.tile([128, TC, T], f16, tag="e", name=f"esd_{b}")
            for jc in range(TC):
                nc.scalar.activation(
                    e_sd[:, jc, :], tmp_sd[:, jc, :], AF.Exp, scale=vd_c[jc]
                )

            # --- attention_out(1) (bilinear) ---
            osb1 = epool.tile([128, TC, H], f16, tag="osb", name=f"osb1_{b}")
            attn_ic(1, e_sb, qn_bf, 0, osb1)
            attn_ic(1, e_sb, qn_bf, 1, osb1)
            attn_dma(1, osb1)

            # sm producer: tanh(qwm[j] - pwm[i]), exp with vm folded
            tmp_sm = epool.tile([128, TC, T], f32, tag="tmp", name=f"tsm_{b}")
            e_sm = epool.tile([128, TC, T], f16, tag="e", name=f"esm_{b}")
            for jc in range(TC):
                nc.scalar.activation(
                    tmp_sm[:, jc, :], pwm_bc, AF.Tanh, bias=qwm_sb[jc], scale=-1.0
                )
                nc.scalar.activation(
                    e_sm[:, jc, :], tmp_sm[:, jc, :], AF.Exp, scale=vm_c[jc]
                )

            # --- attention_out(2) (product) ---
            osb2_ = epool.tile([128, TC, H], f16, tag="osb", name=f"osb2_{b}")
            attn_ic(2, e_sd, qn, 0, osb2_)
            attn_ic(2, e_sd, qn, 1, osb2_)
            attn_dma(2, osb2_)

            # --- attention_out(3) (difference) ---
            osb3 = epool.tile([128, TC, H], f16, tag="osb", name=f"osb3_{b}")
            attn_ic(3, e_sm, qn, 0, osb3)
            attn_ic(3, e_sm, qn, 1, osb3)
            attn_dma(3, osb3)

            if b + 2 < BPC and b + 2 not in pre:
                pre[b + 2] = load_batch(b + 2)

    nc.compile()
    return nc


def _get_program():
    if "nc" not in _CACHE:
        _CACHE["nc"] = _build_program()
    return _CACHE["nc"]


def kernel(**inputs):
    global last_exec_time_ns, last_trace_dir
    import ml_dtypes
    from concourse.bass_utils import run_bass_kernel_spmd

    nc = _get_program()

    f32 = lambda k: np.ascontiguousarray(np.asarray(inputs[k], dtype=np.float32))
    q32 = f32("q")
    p32 = f32("p")
    q16 = q32.astype(np.float16)
    qbf = q32.astype(ml_dtypes.bfloat16)
    p16 = p32.astype(np.float16)
    wc1a = np.ascontiguousarray(
        np.concatenate([f32("Wc1"), f32("Wm")], axis=1).astype(np.float16)
    )
    weights = {
        "Wc1a": wc1a,
        "Wc2": f32("Wc2").astype(np.float16),
        "Wb": f32("Wb").astype(np.float16),
        "Wm16": f32("Wm").astype(np.float16),
        "vc": f32("vc"),
        "Wd": f32("Wd"),
        "vd": f32("vd"),
        "vm": f32("vm"),
    }

    in_maps = []
    for i in range(NCORES):
        m = {
            "q16": q16[i * BPC : (i + 1) * BPC],
            "qbf": qbf[i * BPC : (i + 1) * BPC],
            "p16": p16[i * BPC : (i + 1) * BPC],
        }
        m.update(weights)
        in_maps.append(m)

    trace = bool(int(os.environ.get("BASS_KERNEL_TRACE", "0")))
    kw = {}
    if trace:
        kw.update(trace=True)
        tmpdir = os.environ.get("BASS_KERNEL_TRACE_DIR")
        if tmpdir:
            os.makedirs(tmpdir, exist_ok=True)
            kw.update(tmpdir=tmpdir)
    res = run_bass_kernel_spmd(nc, in_maps, core_ids=list(range(NCORES)), **kw)
    last_exec_time_ns = getattr(res, "exec_time_ns", None)
    results = res.results

    outs = [np.empty((B, T, H), dtype=np.float32) for _ in range(4)]
    for i in range(NCORES):
        o = np.asarray(results[i]["out"], dtype=np.float32)
        for a in range(4):
            outs[a][i * BPC : (i + 1) * BPC] = o[a]
    return tuple(outs)
